# revision 29
# baseline (speedup 1.0000x reference)
"""Trainium2 Bass kernel for nn_CumulativeShadeRegressor.

Model (per sample): per-leaf MLP encoder [L, FD] -> [L, H2] (two gelu
layers), softplus absorb/atten heads, a top-to-bottom exponential
transmittance scan over L, mean-pooling over L, and a small dense head on
[Xg | pooled].

Strategy: data-parallel over B across 8 NeuronCores (32 samples/core).
On each core activations are kept feature-major ([features, tokens]) so
weights are the stationary matmul operand and biases are per-partition for
fused ACT gelu.  Host pre-transposes Xl so no on-device transposes are
needed.  The L-scan is done sample-major ([32 samples, 512]) with the DVE
prefix-scan op; absorb/atten pre-activations reach that layout via a
[2, 512]-per-sample matmul, a DVE PSUM->SBUF copy and an SBUF->SBUF DMA
scatter.  Matmuls run in float32r (full PE rate, ~1e-4 input rounding).
"""
import sys

sys.path.insert(0, "/opt/trn_rl_repo")

import numpy as np
import ml_dtypes

import concourse.bacc as bacc
import concourse.mybir as mybir
import concourse.tile as tile
from concourse.bass_utils import run_bass_kernel_spmd
from concourse.tile import add_dep_helper

B, L, FD, G = 256, 512, 64, 32
H1, H2, DH = 512, 512, 256
NCORES = 8
BL = B // NCORES          # 32 samples per core
NBLK = BL // 4            # 8 blocks of 4 samples

f32 = mybir.dt.float32
f32r = mybir.dt.float32r
bf16 = mybir.dt.bfloat16
AF = mybir.ActivationFunctionType
ALU = mybir.AluOpType
AX = mybir.AxisListType


def _build():
    nc = bacc.Bacc("TRN2", target_bir_lowering=False, debug=False,
                   num_devices=NCORES)

    d = {}
    d["xlt"] = nc.dram_tensor("xlt", [BL * FD, L], bf16, kind="ExternalInput").ap()
    d["xgt"] = nc.dram_tensor("xgt", [G, BL], f32, kind="ExternalInput").ap()
    d["w1s"] = nc.dram_tensor("w1s", [128, H1], bf16, kind="ExternalInput").ap()
    d["w2"] = nc.dram_tensor("w2", [128, 4 * H2], bf16, kind="ExternalInput").ap()
    d["wawt"] = nc.dram_tensor("wawt", [128, 8], f32r, kind="ExternalInput").ap()
    d["wd1g"] = nc.dram_tensor("wd1g", [G, DH], f32, kind="ExternalInput").ap()
    d["wd1p"] = nc.dram_tensor("wd1p", [128, 4 * DH], f32, kind="ExternalInput").ap()
    d["wd2"] = nc.dram_tensor("wd2", [128, 2], f32, kind="ExternalInput").ap()
    d["b1"] = nc.dram_tensor("b1", [128, 4], f32, kind="ExternalInput").ap()
    d["b2"] = nc.dram_tensor("b2", [128, 4], f32, kind="ExternalInput").ap()
    d["bd1"] = nc.dram_tensor("bd1", [128, 2], f32, kind="ExternalInput").ap()
    d["scal"] = nc.dram_tensor("scal", [128, 4], f32, kind="ExternalInput").ap()
    out_d = nc.dram_tensor("out", [BL, 1], f32, kind="ExternalOutput").ap()

    with tile.TileContext(nc) as tc:
        with (
            tc.tile_pool(name="wp", bufs=1) as wp,
            tc.tile_pool(name="persist", bufs=1) as pp,
            tc.tile_pool(name="xp", bufs=4) as xp,
            tc.tile_pool(name="h1p", bufs=10) as h1p,
            tc.tile_pool(name="h2p", bufs=20) as h2p,
            tc.tile_pool(name="awsb", bufs=2) as awsb,
        ):
            w1s_t = wp.tile([128, H1], bf16)
            w2_t = wp.tile([128, 4 * H2], bf16)
            wawt_t = wp.tile([128, 8], f32r)
            xgt_t = wp.tile([G, BL], f32)
            wd1g_t = wp.tile([G, DH], f32)
            wd1p_t = wp.tile([128, 4 * DH], f32)
            wd2_t = wp.tile([128, 2], f32)
            b1_t = wp.tile([128, 4], f32)
            b2_t = wp.tile([128, 4], f32)
            bd1_t = wp.tile([128, 2], f32)
            scal_t = wp.tile([128, 4], f32)
            # first block's inputs land before the bulk of the weights
            nc.sync.dma_start(w1s_t[:], d["w1s"][:])
            nc.sync.dma_start(b1_t[:], d["b1"][:])
            x2_0 = []
            for half in range(2):
                xt = xp.tile([128, L], bf16, name=f"x2_0_{half}", tag="x2")
                nc.sync.dma_start(xt[:], d["xlt"][half * 128:half * 128 + 128, :])
                x2_0.append(xt)
            for nm, t in [("w2", w2_t), ("b2", b2_t), ("wawt", wawt_t),
                          ("xgt", xgt_t), ("wd1g", wd1g_t), ("wd1p", wd1p_t),
                          ("wd2", wd2_t), ("bd1", bd1_t), ("scal", scal_t)]:
                nc.gpsimd.dma_start(t[:], d[nm][:])

            pooled_t = pp.tile([128, 4 * BL], f32)   # [h_part, hc*32 + s]
            absorb_all = pp.tile([BL, L], f32)
            atten_all = pp.tile([BL, L], f32)

            with (
                tc.tile_pool(name="h1ps", bufs=2, space="PSUM") as h1psp,
                tc.tile_pool(name="h2ps", bufs=2, space="PSUM") as h2psp,
                tc.tile_pool(name="awps", bufs=2, space="PSUM") as awpsp,
            ):
                # PE warm-up: back-to-back matmuls on scratch data so the HAM
                # clock gate reaches K=8/8 before (and until) the real work.
                wu_sb = wp.tile([128, 128], f32, name="wu_sb")
                nc.gpsimd.memset(wu_sb[:], 0.0)
                wu_ps = awpsp.tile([128, 128], f32, name="wu_ps", tag="awps")
                for i in range(10):
                    nc.tensor.matmul(wu_ps[:], wu_sb[:], wu_sb[:],
                                     start=True, stop=True)

                for g in range(NBLK):
                    if g == 0:
                        x2 = x2_0
                    else:
                        x2 = []
                        for half in range(2):
                            xt = xp.tile([128, L], bf16, name=f"x2_{g}_{half}", tag="x2")
                            r0 = (g * 4 + half * 2) * FD
                            nc.sync.dma_start(xt[:], d["xlt"][r0:r0 + 128, :])
                            x2.append(xt)

                    # layer 1: [64, L] x [64, 128] -> psum [128, 2*L], gelu
                    h1t = {}
                    for half in range(2):
                        for mc in range(4):
                            ps = h1psp.tile([128, 2 * L], f32,
                                            name=f"h1ps_{g}_{half}_{mc}", tag="h1ps")
                            for sl in range(2):
                                nc.tensor.matmul(
                                    ps[:, sl * L:(sl + 1) * L],
                                    w1s_t[64 * sl:64 * sl + 64, mc * 128:(mc + 1) * 128],
                                    x2[half][64 * sl:64 * sl + 64, :],
                                    start=True, stop=True)
                            t = h1p.tile([128, 2 * L], bf16,
                                         name=f"h1t_{g}_{half}_{mc}", tag="h1t")
                            nc.scalar.activation(t[:], ps[:], AF.Gelu,
                                                 bias=b1_t[:, mc:mc + 1])
                            h1t[half, mc] = t

                    if g == 0:
                        # filler matmuls keep the PE busy through the first
                        # block's ACT pipeline fill so HAM stays at K=8/8
                        for i in range(12):
                            nc.tensor.matmul(wu_ps[:], wu_sb[:], wu_sb[:],
                                             start=True, stop=True)

                    # layer 2: per-sample psum, gelu; pooling sums on DVE
                    h2t = {}
                    for j in range(4):
                        half, sl = j // 2, j % 2
                        s_glob = g * 4 + j
                        for mc in range(4):
                            ps = h2psp.tile([128, L], f32,
                                            name=f"h2ps_{g}_{j}_{mc}", tag="h2ps")
                            for kc in range(4):
                                nc.tensor.matmul(
                                    ps[:],
                                    w2_t[:, kc * H2 + mc * 128:kc * H2 + (mc + 1) * 128],
                                    h1t[half, kc][:, sl * L:(sl + 1) * L],
                                    start=(kc == 0), stop=(kc == 3))
                            t = h2p.tile([128, L], f32r,
                                         name=f"h2t_{g}_{j}_{mc}", tag="h2t")
                            nc.scalar.activation(
                                t[:], ps[:], AF.Gelu, bias=b2_t[:, mc:mc + 1],
                                accum_out=pooled_t[:, mc * BL + s_glob:mc * BL + s_glob + 1])
                            h2t[j, mc] = t

                    # absorb/atten pre-acts [2, L] per sample
                    aw2 = awsb.tile([2, 4 * L], f32, name=f"aw2_{g}", tag="aw2")
                    for j in range(4):
                        ap_ps = awpsp.tile([2, L], f32, name=f"awps_{g}_{j}", tag="awps")
                        for hc in range(4):
                            nc.tensor.matmul(
                                ap_ps[:],
                                wawt_t[:, hc * 2:hc * 2 + 2],
                                h2t[j, hc][:],
                                start=(hc == 0), stop=(hc == 3))
                        nc.vector.tensor_copy(aw2[0:2, j * L:(j + 1) * L], ap_ps[:])
                    nc.sync.dma_start(absorb_all[g * 4:g * 4 + 4, :], aw2[0:1, :])
                    nc.sync.dma_start(atten_all[g * 4:g * 4 + 4, :], aw2[1:2, :])

            # ---- phase 2 ----
            with (
                tc.tile_pool(name="p2ps", bufs=2, space="PSUM") as p2ps,
                tc.tile_pool(name="p2sb", bufs=1) as p2sb,
            ):
                # dense head (still on the gelu activation table)
                d1t = []
                gelu_insts = []
                for mc in range(2):
                    ps = p2ps.tile([128, BL], f32, name=f"d1ps_{mc}", tag="d1ps")
                    nc.tensor.matmul(ps[:], wd1g_t[:, mc * 128:(mc + 1) * 128],
                                     xgt_t[:], start=True, stop=False)
                    for hc in range(4):
                        nc.tensor.matmul(
                            ps[:],
                            wd1p_t[:, hc * DH + mc * 128:hc * DH + (mc + 1) * 128],
                            pooled_t[:, hc * BL:(hc + 1) * BL],
                            start=False, stop=(hc == 3))
                    t = p2sb.tile([128, BL], f32, name=f"d1t_{mc}")
                    gi = nc.scalar.activation(t[:], ps[:], AF.Gelu,
                                              bias=bd1_t[:, mc:mc + 1])
                    gelu_insts.append(gi)
                    d1t.append(t)
                dps = p2ps.tile([BL, 1], f32, name="dps", tag="dps")
                nc.tensor.matmul(dps[:], d1t[0][:], wd2_t[:, 0:1], start=True, stop=False)
                nc.tensor.matmul(dps[:], d1t[1][:], wd2_t[:, 1:2], start=False, stop=True)

                # exp(-softplus(x + b)) = sigmoid(-(x + b)), so the
                # transmittance T[l] = prod_{l'>l} sigmoid(-(atten[l'] + bt))
                # is a reversed inclusive product scan of sigmoids, and
                # softplus(absorb + ba) = -ln(sigmoid(-(absorb + ba))).
                # scal cols 0/1 hold -ba/-bt.
                sgt = p2sb.tile([BL, L], f32, name="sgt")
                si = nc.scalar.activation(sgt[:], atten_all[:], AF.Sigmoid,
                                          bias=scal_t[0:BL, 1:2], scale=-1.0)
                for gi in gelu_insts:
                    add_dep_helper(si.ins, gi.ins, sync=True,
                                   reason="ACT table set order: gelu before sigmoid")
                sga = p2sb.tile([BL, L], f32, name="sga")
                nc.scalar.activation(sga[:], absorb_all[:], AF.Sigmoid,
                                     bias=scal_t[0:BL, 0:1], scale=-1.0)
                # prod[l] = prod_{l'>=l} sgt[l'] via a scan over reversed views
                prod = p2sb.tile([BL, L], f32, name="prod")
                sgt_rev = sgt[:, L - 1::-1]
                prod_rev = prod[:, L - 1::-1]
                nc.vector.tensor_tensor_scan(prod_rev, sgt_rev, sgt_rev, 1.0,
                                             ALU.mult, ALU.bypass)
                nla = p2sb.tile([BL, L], f32, name="nla")  # -softplus(absorb)
                nc.scalar.activation(nla[:], sga[:], AF.Ln)
                # contrib[l] = -softplus(absorb)[l] * T[l], T[l] = prod[l+1]
                contrib = p2sb.tile([BL, L], f32, name="contrib")
                nc.vector.tensor_mul(contrib[:, 0:L - 1], nla[:, 0:L - 1],
                                     prod[:, 1:L])
                nc.vector.tensor_copy(contrib[:, L - 1:L], nla[:, L - 1:L])
                ncap = p2sb.tile([BL, 1], f32, name="ncap")  # -captured
                nc.vector.reduce_sum(ncap[:], contrib[:], axis=AX.X)

                outc = p2sb.tile([BL, 1], f32, name="outc")
                nc.vector.tensor_sub(outc[:], dps[:], ncap[:])
                nc.vector.tensor_scalar_add(outc[:], outc[:], scal_t[0:BL, 2:3])
                nc.sync.dma_start(out_d[:], outc[:])

    nc.compile()
    return nc


_CACHE = {}


def _prep_inputs(inputs):
    f = lambda a: np.ascontiguousarray(np.asarray(a, dtype=np.float32))
    Xg, Xl = f(inputs["Xg"]), f(inputs["Xl"])
    W1, b1 = f(inputs["W1"]), f(inputs["b1"])
    W2, b2 = f(inputs["W2"]), f(inputs["b2"])
    wa, ba = f(inputs["wa"]), f(inputs["ba"])
    wt, bt = f(inputs["wt"]), f(inputs["bt"])
    Wd1, bd1 = f(inputs["Wd1"]), f(inputs["bd1"])
    Wd2, bd2 = f(inputs["Wd2"]), f(inputs["bd2"])

    shared = {
        "w1s": np.ascontiguousarray(np.concatenate([W1, W1], axis=0)).astype(ml_dtypes.bfloat16),
        "w2": np.ascontiguousarray(
            W2.reshape(4, 128, H2).transpose(1, 0, 2).reshape(128, 4 * H2)
        ).astype(ml_dtypes.bfloat16),
        "wawt": np.ascontiguousarray(
            np.concatenate([wa, wt], axis=1).reshape(4, 128, 2)
            .transpose(1, 0, 2).reshape(128, 8)),
        "wd1g": np.ascontiguousarray(Wd1[:G]),
        "wd1p": np.ascontiguousarray(
            (Wd1[G:] / np.float32(L)).reshape(4, 128, DH)
            .transpose(1, 0, 2).reshape(128, 4 * DH)),
        "wd2": np.ascontiguousarray(Wd2.reshape(2, 128).T),
        "b1": np.ascontiguousarray(b1.reshape(4, 128).T),
        "b2": np.ascontiguousarray(b2.reshape(4, 128).T),
        "bd1": np.ascontiguousarray(bd1.reshape(2, 128).T),
    }
    scal = np.zeros((128, 4), np.float32)
    scal[:, 0] = -ba.reshape(-1)[0]
    scal[:, 1] = -bt.reshape(-1)[0]
    scal[:, 2] = bd2.reshape(-1)[0]
    shared["scal"] = scal

    in_maps = []
    for c in range(NCORES):
        s = slice(c * BL, (c + 1) * BL)
        m = dict(shared)
        m["xlt"] = np.ascontiguousarray(
            Xl[s].transpose(0, 2, 1).reshape(BL * FD, L)).astype(ml_dtypes.bfloat16)
        m["xgt"] = np.ascontiguousarray(Xg[s].T)
        in_maps.append(m)
    return in_maps


def _run(inputs, trace=False, tmpdir=None):
    if "nc" not in _CACHE:
        _CACHE["nc"] = _build()
    nc = _CACHE["nc"]
    in_maps = _prep_inputs(inputs)
    res = run_bass_kernel_spmd(nc, in_maps, list(range(NCORES)),
                               trace=trace, tmpdir=tmpdir)
    out = np.concatenate([res.results[c]["out"] for c in range(NCORES)], axis=0)
    return out.astype(np.float32), res


def kernel(**inputs) -> np.ndarray:
    out, _ = _run(inputs)
    return out


# revision 30
# speedup vs baseline: 1.0112x; 1.0112x over previous
"""Trainium2 Bass kernel for nn_CumulativeShadeRegressor.

Model (per sample): per-leaf MLP encoder [L, FD] -> [L, H2] (two gelu
layers), softplus absorb/atten heads, a top-to-bottom exponential
transmittance scan over L, mean-pooling over L, and a small dense head on
[Xg | pooled].

Strategy: data-parallel over B across 8 NeuronCores (32 samples/core).
On each core activations are kept feature-major ([features, tokens]) so
weights are the stationary matmul operand and biases are per-partition for
fused ACT gelu.  Host pre-transposes Xl so no on-device transposes are
needed.  The L-scan is done sample-major ([32 samples, 512]) with the DVE
prefix-scan op; absorb/atten pre-activations reach that layout via a
[2, 512]-per-sample matmul, a DVE PSUM->SBUF copy and an SBUF->SBUF DMA
scatter.  Matmuls run in float32r (full PE rate, ~1e-4 input rounding).
"""
import sys

sys.path.insert(0, "/opt/trn_rl_repo")

import numpy as np
import ml_dtypes

import concourse.bacc as bacc
import concourse.mybir as mybir
import concourse.tile as tile
from concourse.bass_utils import run_bass_kernel_spmd
from concourse.tile import add_dep_helper

B, L, FD, G = 256, 512, 64, 32
H1, H2, DH = 512, 512, 256
NCORES = 8
BL = B // NCORES          # 32 samples per core
NBLK = BL // 4            # 8 blocks of 4 samples

f32 = mybir.dt.float32
f32r = mybir.dt.float32r
bf16 = mybir.dt.bfloat16
AF = mybir.ActivationFunctionType
ALU = mybir.AluOpType
AX = mybir.AxisListType


def _build():
    nc = bacc.Bacc("TRN2", target_bir_lowering=False, debug=False,
                   num_devices=NCORES)

    d = {}
    d["xlt"] = nc.dram_tensor("xlt", [BL * FD, L], bf16, kind="ExternalInput").ap()
    d["xgt"] = nc.dram_tensor("xgt", [G, BL], f32, kind="ExternalInput").ap()
    d["w1s"] = nc.dram_tensor("w1s", [128, H1], bf16, kind="ExternalInput").ap()
    d["w2"] = nc.dram_tensor("w2", [128, 4 * H2], bf16, kind="ExternalInput").ap()
    d["wawt"] = nc.dram_tensor("wawt", [128, 8], f32r, kind="ExternalInput").ap()
    d["wd1g"] = nc.dram_tensor("wd1g", [G, DH], f32, kind="ExternalInput").ap()
    d["wd1p"] = nc.dram_tensor("wd1p", [128, 4 * DH], f32, kind="ExternalInput").ap()
    d["wd2"] = nc.dram_tensor("wd2", [128, 2], f32, kind="ExternalInput").ap()
    d["b1"] = nc.dram_tensor("b1", [128, 4], f32, kind="ExternalInput").ap()
    d["b2"] = nc.dram_tensor("b2", [128, 4], f32, kind="ExternalInput").ap()
    d["bd1"] = nc.dram_tensor("bd1", [128, 2], f32, kind="ExternalInput").ap()
    d["scal"] = nc.dram_tensor("scal", [128, 4], f32, kind="ExternalInput").ap()
    out_d = nc.dram_tensor("out", [BL, 1], f32, kind="ExternalOutput").ap()

    with tile.TileContext(nc) as tc:
        with (
            tc.tile_pool(name="wp", bufs=1) as wp,
            tc.tile_pool(name="persist", bufs=1) as pp,
            tc.tile_pool(name="xp", bufs=4) as xp,
            tc.tile_pool(name="h1p", bufs=10) as h1p,
            tc.tile_pool(name="h2p", bufs=20) as h2p,
            tc.tile_pool(name="awsb", bufs=2) as awsb,
        ):
            w1s_t = wp.tile([128, H1], bf16)
            w2_t = wp.tile([128, 4 * H2], bf16)
            wawt_t = wp.tile([128, 8], f32r)
            xgt_t = wp.tile([G, BL], f32)
            wd1g_t = wp.tile([G, DH], f32)
            wd1p_t = wp.tile([128, 4 * DH], f32)
            wd2_t = wp.tile([128, 2], f32)
            b1_t = wp.tile([128, 4], f32)
            b2_t = wp.tile([128, 4], f32)
            bd1_t = wp.tile([128, 2], f32)
            scal_t = wp.tile([128, 4], f32)
            # first block's inputs land before the bulk of the weights
            nc.sync.dma_start(w1s_t[:], d["w1s"][:])
            nc.sync.dma_start(b1_t[:], d["b1"][:])
            x2_0 = []
            for half in range(2):
                xt = xp.tile([128, L], bf16, name=f"x2_0_{half}", tag="x2")
                nc.sync.dma_start(xt[:], d["xlt"][half * 128:half * 128 + 128, :])
                x2_0.append(xt)
            for nm, t in [("w2", w2_t), ("b2", b2_t), ("wawt", wawt_t)]:
                nc.sync.dma_start(t[:], d[nm][:])
            for nm, t in [("xgt", xgt_t), ("wd1g", wd1g_t), ("wd1p", wd1p_t),
                          ("wd2", wd2_t), ("bd1", bd1_t), ("scal", scal_t)]:
                nc.gpsimd.dma_start(t[:], d[nm][:])

            pooled_t = pp.tile([128, 4 * BL], f32)   # [h_part, hc*32 + s]
            absorb_all = pp.tile([BL, L], f32)
            atten_all = pp.tile([BL, L], f32)

            with (
                tc.tile_pool(name="h1ps", bufs=2, space="PSUM") as h1psp,
                tc.tile_pool(name="h2ps", bufs=2, space="PSUM") as h2psp,
                tc.tile_pool(name="awps", bufs=2, space="PSUM") as awpsp,
            ):
                # PE warm-up: back-to-back matmuls on scratch data so the HAM
                # clock gate reaches K=8/8 before (and until) the real work.
                wu_sb = wp.tile([128, 128], f32, name="wu_sb")
                nc.gpsimd.memset(wu_sb[:], 0.0)
                wu_ps = awpsp.tile([128, 128], f32, name="wu_ps", tag="awps")
                for i in range(10):
                    nc.tensor.matmul(wu_ps[:], wu_sb[:], wu_sb[:],
                                     start=True, stop=True)

                for g in range(NBLK):
                    if g == 0:
                        x2 = x2_0
                    else:
                        x2 = []
                        for half in range(2):
                            xt = xp.tile([128, L], bf16, name=f"x2_{g}_{half}", tag="x2")
                            r0 = (g * 4 + half * 2) * FD
                            nc.sync.dma_start(xt[:], d["xlt"][r0:r0 + 128, :])
                            x2.append(xt)

                    # layer 1: [64, L] x [64, 128] -> psum [128, 2*L], gelu
                    h1t = {}
                    for half in range(2):
                        for mc in range(4):
                            ps = h1psp.tile([128, 2 * L], f32,
                                            name=f"h1ps_{g}_{half}_{mc}", tag="h1ps")
                            for sl in range(2):
                                nc.tensor.matmul(
                                    ps[:, sl * L:(sl + 1) * L],
                                    w1s_t[64 * sl:64 * sl + 64, mc * 128:(mc + 1) * 128],
                                    x2[half][64 * sl:64 * sl + 64, :],
                                    start=True, stop=True)
                            t = h1p.tile([128, 2 * L], bf16,
                                         name=f"h1t_{g}_{half}_{mc}", tag="h1t")
                            nc.scalar.activation(t[:], ps[:], AF.Gelu,
                                                 bias=b1_t[:, mc:mc + 1])
                            h1t[half, mc] = t

                    if g == 0:
                        # filler matmuls keep the PE busy through the first
                        # block's ACT pipeline fill so HAM stays at K=8/8
                        for i in range(12):
                            nc.tensor.matmul(wu_ps[:], wu_sb[:], wu_sb[:],
                                             start=True, stop=True)

                    # layer 2: per-sample psum, gelu; pooling sums on DVE
                    h2t = {}
                    for j in range(4):
                        half, sl = j // 2, j % 2
                        s_glob = g * 4 + j
                        for mc in range(4):
                            ps = h2psp.tile([128, L], f32,
                                            name=f"h2ps_{g}_{j}_{mc}", tag="h2ps")
                            for kc in range(4):
                                nc.tensor.matmul(
                                    ps[:],
                                    w2_t[:, kc * H2 + mc * 128:kc * H2 + (mc + 1) * 128],
                                    h1t[half, kc][:, sl * L:(sl + 1) * L],
                                    start=(kc == 0), stop=(kc == 3))
                            t = h2p.tile([128, L], f32r,
                                         name=f"h2t_{g}_{j}_{mc}", tag="h2t")
                            nc.scalar.activation(
                                t[:], ps[:], AF.Gelu, bias=b2_t[:, mc:mc + 1],
                                accum_out=pooled_t[:, mc * BL + s_glob:mc * BL + s_glob + 1])
                            h2t[j, mc] = t

                    # absorb/atten pre-acts [2, L] per sample
                    aw2 = awsb.tile([2, 4 * L], f32, name=f"aw2_{g}", tag="aw2")
                    for j in range(4):
                        ap_ps = awpsp.tile([2, L], f32, name=f"awps_{g}_{j}", tag="awps")
                        for hc in range(4):
                            nc.tensor.matmul(
                                ap_ps[:],
                                wawt_t[:, hc * 2:hc * 2 + 2],
                                h2t[j, hc][:],
                                start=(hc == 0), stop=(hc == 3))
                        nc.vector.tensor_copy(aw2[0:2, j * L:(j + 1) * L], ap_ps[:])
                    nc.sync.dma_start(absorb_all[g * 4:g * 4 + 4, :], aw2[0:1, :])
                    nc.sync.dma_start(atten_all[g * 4:g * 4 + 4, :], aw2[1:2, :])

            # ---- phase 2 ----
            with (
                tc.tile_pool(name="p2ps", bufs=2, space="PSUM") as p2ps,
                tc.tile_pool(name="p2sb", bufs=1) as p2sb,
            ):
                # dense head (still on the gelu activation table)
                d1t = []
                gelu_insts = []
                for mc in range(2):
                    ps = p2ps.tile([128, BL], f32, name=f"d1ps_{mc}", tag="d1ps")
                    nc.tensor.matmul(ps[:], wd1g_t[:, mc * 128:(mc + 1) * 128],
                                     xgt_t[:], start=True, stop=False)
                    for hc in range(4):
                        nc.tensor.matmul(
                            ps[:],
                            wd1p_t[:, hc * DH + mc * 128:hc * DH + (mc + 1) * 128],
                            pooled_t[:, hc * BL:(hc + 1) * BL],
                            start=False, stop=(hc == 3))
                    t = p2sb.tile([128, BL], f32, name=f"d1t_{mc}")
                    gi = nc.scalar.activation(t[:], ps[:], AF.Gelu,
                                              bias=bd1_t[:, mc:mc + 1])
                    gelu_insts.append(gi)
                    d1t.append(t)
                dps = p2ps.tile([BL, 1], f32, name="dps", tag="dps")
                nc.tensor.matmul(dps[:], d1t[0][:], wd2_t[:, 0:1], start=True, stop=False)
                nc.tensor.matmul(dps[:], d1t[1][:], wd2_t[:, 1:2], start=False, stop=True)

                # exp(-softplus(x + b)) = sigmoid(-(x + b)), so the
                # transmittance T[l] = prod_{l'>l} sigmoid(-(atten[l'] + bt))
                # is a reversed inclusive product scan of sigmoids, and
                # softplus(absorb + ba) = -ln(sigmoid(-(absorb + ba))).
                # scal cols 0/1 hold -ba/-bt.
                sgt = p2sb.tile([BL, L], f32, name="sgt")
                si = nc.scalar.activation(sgt[:], atten_all[:], AF.Sigmoid,
                                          bias=scal_t[0:BL, 1:2], scale=-1.0)
                for gi in gelu_insts:
                    add_dep_helper(si.ins, gi.ins, sync=True,
                                   reason="ACT table set order: gelu before sigmoid")
                sga = p2sb.tile([BL, L], f32, name="sga")
                nc.scalar.activation(sga[:], absorb_all[:], AF.Sigmoid,
                                     bias=scal_t[0:BL, 0:1], scale=-1.0)
                # prod[l] = prod_{l'>=l} sgt[l'] via a scan over reversed views
                prod = p2sb.tile([BL, L], f32, name="prod")
                sgt_rev = sgt[:, L - 1::-1]
                prod_rev = prod[:, L - 1::-1]
                nc.vector.tensor_tensor_scan(prod_rev, sgt_rev, sgt_rev, 1.0,
                                             ALU.mult, ALU.bypass)
                nla = p2sb.tile([BL, L], f32, name="nla")  # -softplus(absorb)
                nc.scalar.activation(nla[:], sga[:], AF.Ln)
                # contrib[l] = -softplus(absorb)[l] * T[l], T[l] = prod[l+1]
                contrib = p2sb.tile([BL, L], f32, name="contrib")
                nc.vector.tensor_mul(contrib[:, 0:L - 1], nla[:, 0:L - 1],
                                     prod[:, 1:L])
                nc.vector.tensor_copy(contrib[:, L - 1:L], nla[:, L - 1:L])
                ncap = p2sb.tile([BL, 1], f32, name="ncap")  # -captured
                nc.vector.reduce_sum(ncap[:], contrib[:], axis=AX.X)

                outc = p2sb.tile([BL, 1], f32, name="outc")
                nc.vector.tensor_sub(outc[:], dps[:], ncap[:])
                nc.vector.tensor_scalar_add(outc[:], outc[:], scal_t[0:BL, 2:3])
                nc.sync.dma_start(out_d[:], outc[:])

    nc.compile()
    return nc


_CACHE = {}


def _prep_inputs(inputs):
    f = lambda a: np.ascontiguousarray(np.asarray(a, dtype=np.float32))
    Xg, Xl = f(inputs["Xg"]), f(inputs["Xl"])
    W1, b1 = f(inputs["W1"]), f(inputs["b1"])
    W2, b2 = f(inputs["W2"]), f(inputs["b2"])
    wa, ba = f(inputs["wa"]), f(inputs["ba"])
    wt, bt = f(inputs["wt"]), f(inputs["bt"])
    Wd1, bd1 = f(inputs["Wd1"]), f(inputs["bd1"])
    Wd2, bd2 = f(inputs["Wd2"]), f(inputs["bd2"])

    shared = {
        "w1s": np.ascontiguousarray(np.concatenate([W1, W1], axis=0)).astype(ml_dtypes.bfloat16),
        "w2": np.ascontiguousarray(
            W2.reshape(4, 128, H2).transpose(1, 0, 2).reshape(128, 4 * H2)
        ).astype(ml_dtypes.bfloat16),
        "wawt": np.ascontiguousarray(
            np.concatenate([wa, wt], axis=1).reshape(4, 128, 2)
            .transpose(1, 0, 2).reshape(128, 8)),
        "wd1g": np.ascontiguousarray(Wd1[:G]),
        "wd1p": np.ascontiguousarray(
            (Wd1[G:] / np.float32(L)).reshape(4, 128, DH)
            .transpose(1, 0, 2).reshape(128, 4 * DH)),
        "wd2": np.ascontiguousarray(Wd2.reshape(2, 128).T),
        "b1": np.ascontiguousarray(b1.reshape(4, 128).T),
        "b2": np.ascontiguousarray(b2.reshape(4, 128).T),
        "bd1": np.ascontiguousarray(bd1.reshape(2, 128).T),
    }
    scal = np.zeros((128, 4), np.float32)
    scal[:, 0] = -ba.reshape(-1)[0]
    scal[:, 1] = -bt.reshape(-1)[0]
    scal[:, 2] = bd2.reshape(-1)[0]
    shared["scal"] = scal

    in_maps = []
    for c in range(NCORES):
        s = slice(c * BL, (c + 1) * BL)
        m = dict(shared)
        m["xlt"] = np.ascontiguousarray(
            Xl[s].transpose(0, 2, 1).reshape(BL * FD, L)).astype(ml_dtypes.bfloat16)
        m["xgt"] = np.ascontiguousarray(Xg[s].T)
        in_maps.append(m)
    return in_maps


def _run(inputs, trace=False, tmpdir=None):
    if "nc" not in _CACHE:
        _CACHE["nc"] = _build()
    nc = _CACHE["nc"]
    in_maps = _prep_inputs(inputs)
    res = run_bass_kernel_spmd(nc, in_maps, list(range(NCORES)),
                               trace=trace, tmpdir=tmpdir)
    out = np.concatenate([res.results[c]["out"] for c in range(NCORES)], axis=0)
    return out.astype(np.float32), res


def kernel(**inputs) -> np.ndarray:
    out, _ = _run(inputs)
    return out
